# revision 1
# baseline (speedup 1.0000x reference)
"""Trainium2 Bass kernel for nn_DCGAN_C (DCGAN conv backbone + relation network).

Self-contained: takes FULL inputs (as produced by the problem's setup_inputs),
shards the batch of 32 across 8 NeuronCores (4 samples/core), runs one SPMD
Bass/Tile NEFF (conv backbone with cross-core BatchNorm AllReduce + factorized
relation-network), gathers [32, 1] fp32 output.

Decomposition notes:
- conv1 (12->64, 4x4 s2 p1): im2col over (dx-parity j, dy, ic) -> K=96, 2
  matmuls; the lhsT has the 64 output channels DUPLICATED across both column
  halves so one matmul emits the result on partitions 0-63 AND 64-127 (the
  second, column-shifted copy that conv2's K=128 dx-parity packing needs).
- conv2 (64->128): dx-parity packed K=128, 8 accumulating matmuls.
- conv3 (128->256): 16 direct tap matmuls, K=128, strided rhs APs.
- BatchNorm (training mode, global batch stats): per-core partial sum/sumsq via
  fused ACT accum_out, tiny AllReduce over the 8 cores, affine+leaky-relu
  (Prelu, exact on HW) applied straight from PSUM-resident conv outputs.
- RN layer 1 factorized: pairs@gW1 = A[j] + C[i]; A/C are 64x256 per sample;
  the 4096-pair pre-activation is rebuilt on the PE with a constant 0/1
  selector matrix G (K=128).
- RN layers 2-4: dense [4096,256]x[256,256] per sample in fp16, PSUM fp32.
- pooled sum over 4096 pairs: fused ACT accum_out columns + tiny reduce
  (DVE tensor_scalar accum_out is broken on HW - measured garbage).
- f-MLP: batched over the core's 4 samples (N=4 matmuls).
"""
import os
import numpy as np
from contextlib import ExitStack

NCORES = 8
BPC = 4          # samples per core
NUM = 64         # spatial positions (8x8)
AUX = 10
EPS = 1e-5

_CACHE = {}

# fp16 const-pack layout: (name, rows, width)
_P16 = [
    ("c1w", 48, 4 * 128),        # [dy*12+ic][dx*128 + oc(dup both halves)]
    ("c2w", 128, 8 * 128),       # [j*64+ic][(dy*2+m)*128 + oc]
    ("c3w", 128, 32 * 128),      # [ic][((dy*4+dx)*2+mc)*128 + oc']
    ("w1t", 128, 2 * 256),       # [p][kc*256 + c]
    ("w1b", 128, 2 * 256),
    ("exA", 2, 256),
    ("exC", 11, 256),
    ("aro", 2, 64),              # row0 arange, row1 ones
    ("gw2", 128, 4 * 128),       # [(kc*2+mc)*128 + c]
    ("gw3", 128, 4 * 128),
    ("gw4", 128, 4 * 128),
    ("fw1", 128, 4 * 128),
    ("fw2", 128, 4 * 128),
    ("fw3", 128, 2),
]
_P16_OFF = {}
_off = 0
for _nm, _r, _w in _P16:
    _P16_OFF[_nm] = _off
    _off += _w
P16_W = _off
# fp32 pack [32, 176]: consts rows 0-19 cols 0-128; ident [0:32,128:160];
# fb3 [0,160]; aux [0:bpc, 161:171]
P32_W = 176


def _host_pack(inputs):
    f16 = np.float16
    w1 = inputs['conv1_w'].astype(np.float32)
    w2 = inputs['conv2_w'].astype(np.float32)
    w3 = inputs['conv3_w'].astype(np.float32)
    gW1 = inputs['gW1'].astype(np.float32)
    gb1 = inputs['gb1'].astype(np.float32)

    pk = np.zeros((128, P16_W), f16)

    def put(nm, arr):
        o = _P16_OFF[nm]
        r, w = arr.shape
        pk[0:r, o:o + w] = arr.astype(f16)

    # conv1: c1w[dy*12+ic, dx*128+oc] with oc duplicated in cols 64..127
    c1 = np.zeros((48, 4 * 128), np.float32)
    t = np.transpose(w1, (3, 2, 1, 0))   # [dx, dy, ic, oc]
    for dx in range(4):
        blk = t[dx].reshape(48, 64)
        c1[:, dx * 128:dx * 128 + 64] = blk
        c1[:, dx * 128 + 64:dx * 128 + 128] = blk
    put("c1w", c1)

    c2 = np.zeros((128, 8 * 128), np.float32)
    for dy in range(4):
        for m in range(2):
            o = (dy * 2 + m) * 128
            for j in range(2):
                c2[j * 64:(j + 1) * 64, o:o + 128] = w2[:, :, dy, 2 * m + j].T
    put("c2w", c2)

    c3 = np.zeros((128, 32 * 128), np.float32)
    for dy in range(4):
        for dx in range(4):
            for mc in range(2):
                o = ((dy * 4 + dx) * 2 + mc) * 128
                c3[:, o:o + 128] = w3[mc * 128:(mc + 1) * 128, :, dy, dx].T
    put("c3w", c3)

    w1t = np.zeros((128, 512), np.float32)
    w1b = np.zeros((128, 512), np.float32)
    for kc in range(2):
        w1t[:, kc * 256:(kc + 1) * 256] = gW1[kc * 128:(kc + 1) * 128]
        w1b[:, kc * 256:(kc + 1) * 256] = gW1[257 + kc * 128:257 + (kc + 1) * 128]
    put("w1t", w1t)
    put("w1b", w1b)
    put("exA", np.stack([gW1[256], gb1], 0))
    put("exC", np.concatenate([gW1[513:523], gW1[523:524]], 0))
    put("aro", np.stack([np.arange(64, dtype=np.float32),
                         np.ones(64, np.float32)], 0))

    def sq(nm, W):
        o = np.zeros((128, 512), np.float32)
        for kc in range(2):
            for mc in range(2):
                o[:, (kc * 2 + mc) * 128:(kc * 2 + mc + 1) * 128] = \
                    W[kc * 128:(kc + 1) * 128, mc * 128:(mc + 1) * 128]
        put(nm, o)

    sq("gw2", inputs['gW2']); sq("gw3", inputs['gW3']); sq("gw4", inputs['gW4'])
    sq("fw1", inputs['fW1']); sq("fw2", inputs['fW2'])
    f3 = np.zeros((128, 2), np.float32)
    for kc in range(2):
        f3[:, kc] = inputs['fW3'][kc * 128:(kc + 1) * 128, 0]
    put("fw3", f3)

    G = np.zeros((128, NUM * NUM), f16)
    ii, jj = np.meshgrid(np.arange(NUM), np.arange(NUM), indexing='ij')
    p = (ii * NUM + jj).ravel()
    G[jj.ravel(), p] = 1.0
    G[64 + ii.ravel(), p] = 1.0

    pk32 = np.zeros((32, P32_W), np.float32)
    pk32[0, 0:128] = inputs['bn2_g']; pk32[1, 0:128] = inputs['bn2_b']
    pk32[2, 0:128] = inputs['bn3_g'][:128]; pk32[3, 0:128] = inputs['bn3_g'][128:]
    pk32[4, 0:128] = inputs['bn3_b'][:128]; pk32[5, 0:128] = inputs['bn3_b'][128:]
    pk32[6, 0:128] = inputs['gb2'][:128]; pk32[7, 0:128] = inputs['gb2'][128:]
    pk32[8, 0:128] = inputs['gb3'][:128]; pk32[9, 0:128] = inputs['gb3'][128:]
    pk32[10, 0:128] = inputs['gb4'][:128]; pk32[11, 0:128] = inputs['gb4'][128:]
    pk32[12, 0:128] = inputs['fb1'][:128]; pk32[13, 0:128] = inputs['fb1'][128:]
    pk32[14, 0:128] = inputs['fb2'][:128]; pk32[15, 0:128] = inputs['fb2'][128:]
    pk32[0:32, 128:160] = np.eye(32, dtype=np.float32)
    pk32[0, 160] = float(np.asarray(inputs['fb3']).reshape(-1)[0])
    return pk, G, pk32


def _build(num_devices, bpc, debug=False):
    import concourse.bacc as bacc
    import concourse.mybir as mybir
    import concourse.tile as tile

    f16 = mybir.dt.float16
    f32 = mybir.dt.float32
    AF = mybir.ActivationFunctionType
    OP = mybir.AluOpType

    N2 = num_devices * bpc * 256
    N3 = num_devices * bpc * 64

    nc = bacc.Bacc(None, target_bir_lowering=False, num_devices=num_devices)

    img = nc.dram_tensor("img", [bpc, 12, 64, 64], f32, kind="ExternalInput")
    pk16 = nc.dram_tensor("pk16", [128, P16_W], f16, kind="ExternalInput")
    Gd = nc.dram_tensor("Gd", [128, NUM * NUM], f16, kind="ExternalInput")
    pk32 = nc.dram_tensor("pk32", [32, P32_W], f32, kind="ExternalInput")
    out = nc.dram_tensor("out", [1, bpc], f32, kind="ExternalOutput")
    dbg = {}
    if debug:
        for nm, shape, dt_ in (
                ("dbg_h1", [128, bpc, 34, 34], f16),
                ("dbg_s1", [128, 2], f32),
                ("dbg_h2", [128, bpc, 18, 18], f16),
                ("dbg_s2", [128, 4], f32),
                ("dbg_enc", [128, 2, bpc, 64], f16),
                ("dbg_ac", [128, 256], f16),
                ("dbg_g1", [128, 2, 4096], f16),
                ("dbg_g3", [128, 2, 4096], f16),
                ("dbg_pool", [128, bpc, 2], f32),
        ):
            dbg[nm] = nc.dram_tensor(nm, shape, dt_, kind="ExternalOutput")

    cc1_in = nc.dram_tensor("cc1_in", [128, 2], f32)
    cc1_out = nc.dram_tensor("cc1_out", [128, 2], f32, addr_space="Shared")
    cc2_in = nc.dram_tensor("cc2_in", [128, 4], f32)
    cc2_out = nc.dram_tensor("cc2_out", [128, 4], f32, addr_space="Shared")
    groups = [list(range(num_devices))]

    with tile.TileContext(nc) as tc, ExitStack() as ctx:
        cw = ctx.enter_context(tc.tile_pool(name="cw", bufs=1))
        st = ctx.enter_context(tc.tile_pool(name="st", bufs=1))
        sm = ctx.enter_context(tc.tile_pool(name="sm", bufs=2))
        trash = ctx.enter_context(tc.tile_pool(name="trash", bufs=4))
        big = ctx.enter_context(tc.tile_pool(name="big", bufs=3, space="PSUM"))
        hold = ctx.enter_context(tc.tile_pool(name="hold", bufs=2, space="PSUM"))

        PK = cw.tile([128, P16_W], f16, name="PK")
        nc.sync.dma_start(out=PK, in_=pk16.ap())
        G_s = cw.tile([128, NUM * NUM], f16, name="G_s")
        nc.sync.dma_start(out=G_s, in_=Gd.ap())
        P32 = cw.tile([32, P32_W], f32, name="P32")
        nc.sync.dma_start(out=P32, in_=pk32.ap())

        def pk_at(nm, o0, width, rows=128, r0=0):
            o = _P16_OFF[nm] + o0
            return PK[r0:r0 + rows, o:o + width]

        csb = P32[0:20, 0:128]
        id_s = P32[0:32, 128:160]
        fb3_s = P32[0:1, 160:161]
        asb = P32[0:bpc, 161:161 + AUX]

        # consts transpose -> CONST_T [128, 20]
        CONST_T = st.tile([128, 20], f32)
        ptc = hold.tile([128, 512], f32, tag="hold")
        nc.tensor.transpose(ptc[:, 0:20], csb, id_s[0:20, 0:20])
        nc.vector.tensor_copy(CONST_T, ptc[:, 0:20])
        # aux transpose -> AUXT [10, bpc]
        AUXT = st.tile([AUX, bpc], f16)
        pta = hold.tile([128, 512], f32, tag="hold")
        nc.tensor.transpose(pta[0:AUX, 0:bpc], asb, id_s[0:bpc, 0:bpc])
        nc.vector.tensor_copy(AUXT, pta[0:AUX, 0:bpc])

        def col(i):
            return CONST_T[:, i:i + 1]

        def relu_bias(use_act, dst, psum, bias_col, accum=None):
            if use_act:
                nc.scalar.activation(out=dst, in_=psum, func=AF.Relu,
                                     bias=bias_col if bias_col is not None else 0.0,
                                     accum_out=accum)
            else:
                assert accum is None
                nc.vector.tensor_scalar(
                    out=dst, in0=psum,
                    scalar1=bias_col if bias_col is not None else 0.0,
                    scalar2=0.0, op0=OP.add, op1=OP.max)

        def bn_lrelu(dst, psum, a_col, d_col):
            nc.scalar.activation(out=dst, in_=psum, func=AF.Prelu,
                                 bias=d_col, scale=a_col, alpha=0.2)

        # ---- conv1: per-sample IM tiles [48=(dy,ic), 32oy, 64u] ----
        # IM[dy*12+ic, oy, u] = img[s, ic, 2oy+dy-1, u]; edge taps handled by
        # region-restricted PSUM accumulation (no column padding needed).
        H1 = st.tile([128, bpc, 34, 34], f16)
        nc.vector.memset(H1, 0.0)
        H2 = st.tile([128, bpc, 18, 18], f16)
        nc.vector.memset(H2, 0.0)

        imgv = img.rearrange("s c (r2 a) w -> c s r2 a w", a=2)
        dma_engines = [nc.sync, nc.gpsimd, nc.scalar]
        imp = ctx.enter_context(tc.tile_pool(name="imp", bufs=2))
        # tap order: full-coverage taps first (dx=1 carries start=True)
        DX_ORDER = (1, 2, 0, 3)

        for s in range(bpc):
            imf = imp.tile([48, 32, 64], f32, tag="imf")
            # edge rows (32-aligned partition base required): zero rows 0 and
            # 31 across all dy groups; the DMAs below overwrite the valid ones
            nc.vector.memset(imf[:, 0:1, :], 0.0)
            nc.vector.memset(imf[:, 31:32, :], 0.0)
            for dy in range(4):
                oy0, nrows = {0: (1, 31), 1: (0, 32), 2: (0, 32), 3: (0, 31)}[dy]
                r20, a = {0: (0, 1), 1: (0, 0), 2: (0, 1), 3: (1, 0)}[dy]
                dma_engines[(s * 4 + dy) % 3].dma_start(
                    out=imf[dy * 12:(dy + 1) * 12, oy0:oy0 + nrows, :],
                    in_=imgv[:, s, r20:r20 + nrows, a, :],
                )
            imh = imp.tile([48, 32, 64], f16, tag="imh")
            nc.vector.tensor_copy(imh[:, 0:16], imf[:, 0:16])
            nc.scalar.copy(imh[:, 16:32], imf[:, 16:32])
            imv = imh.rearrange("p y (v b) -> p y v b", b=2)  # u = 2v + b

            pt = big.tile([128, 1024], f32, tag="mm")
            ptv = pt.rearrange("p (y x) -> p y x", x=32)
            for oyh in range(2):
                ys = slice(oyh * 16, (oyh + 1) * 16)
                for dx in DX_ORDER:
                    # u = 2ox + dx - 1 = 2v + b
                    if dx == 1:
                        xs, vs, b = slice(0, 32), slice(0, 32), 0
                    elif dx == 2:
                        xs, vs, b = slice(0, 32), slice(0, 32), 1
                    elif dx == 0:
                        xs, vs, b = slice(1, 32), slice(0, 31), 1
                    else:  # dx == 3
                        xs, vs, b = slice(0, 31), slice(1, 32), 0
                    nc.tensor.matmul(
                        ptv[:, ys, xs],
                        pk_at("c1w", dx * 128, 128, rows=48),
                        imv[0:48, ys, vs, b],
                        start=(dx == 1), stop=(dx == 3),
                    )
            nc.scalar.activation(out=H1[0:64, s, 1:33, 1:33], in_=ptv[0:64],
                                 func=AF.Prelu, alpha=0.2)
            nc.scalar.activation(out=H1[64:128, s, 1:33, 0:32], in_=ptv[64:128],
                                 func=AF.Prelu, alpha=0.2)

        # ---------------- conv2 + BN2 ----------------
        H1v = H1.rearrange("p s (r a) (v b) -> p s r a v b", a=2, b=2)
        c2p = []
        for nt in range(2):
            pt = hold.tile([128, 512], f32, tag="hold")
            c2p.append(pt)
            k = 0
            for dy in range(4):
                for m in range(2):
                    nc.tensor.matmul(
                        pt,
                        pk_at("c2w", (dy * 2 + m) * 128, 128),
                        H1v[:, 2 * nt:2 * nt + 2, dy // 2:dy // 2 + 16, dy % 2,
                            m:m + 16, 0],
                        start=(k == 0), stop=(k == 7),
                    )
                    k += 1
        s2c = sm.tile([128, 4], f32, tag="s2c")
        for nt in range(2):
            tb1 = trash.tile([128, 512], f16, tag="tr")
            nc.scalar.activation(out=tb1, in_=c2p[nt], func=AF.Square,
                                 accum_out=s2c[:, 2 + nt:3 + nt])
            tb2 = trash.tile([128, 512], f16, tag="tr")
            nc.scalar.activation(out=tb2, in_=c2p[nt], func=AF.Identity,
                                 accum_out=s2c[:, nt:nt + 1])
        AR1 = sm.tile([128, 2], f32, tag="ar1")
        nc.vector.tensor_reduce(out=AR1[:, 0:1], in_=s2c[:, 0:2],
                                axis=mybir.AxisListType.X, op=OP.add)
        nc.vector.tensor_reduce(out=AR1[:, 1:2], in_=s2c[:, 2:4],
                                axis=mybir.AxisListType.X, op=OP.add)
        nc.gpsimd.dma_start(out=cc1_in[:, :], in_=AR1)
        nc.gpsimd.collective_compute(
            "AllReduce", OP.add, replica_groups=groups,
            ins=[cc1_in[:, :]], outs=[cc1_out[:, :]])
        S1 = sm.tile([128, 2], f32, tag="s1")
        nc.gpsimd.dma_start(out=S1, in_=cc1_out[:, :])

        def bn_finalize(S, scol, qcol, inv_n, g_col, b_col):
            mean = sm.tile([128, 1], f32, tag="bnm")
            nc.vector.tensor_scalar(out=mean, in0=S[:, scol:scol + 1],
                                    scalar1=inv_n, scalar2=None, op0=OP.mult)
            var = sm.tile([128, 1], f32, tag="bnv")
            nc.vector.tensor_scalar(out=var, in0=S[:, qcol:qcol + 1],
                                    scalar1=inv_n, scalar2=None, op0=OP.mult)
            msq = sm.tile([128, 1], f32, tag="bnq")
            nc.vector.tensor_tensor(out=msq, in0=mean, in1=mean, op=OP.mult)
            nc.vector.tensor_tensor(out=var, in0=var, in1=msq, op=OP.subtract)
            std = sm.tile([128, 1], f32, tag="bns")
            epsb = sm.tile([128, 1], f32, tag="bne")
            nc.vector.memset(epsb, EPS)
            nc.scalar.activation(out=std, in_=var, func=AF.Sqrt, bias=epsb[:, 0:1])
            rstd = sm.tile([128, 1], f32, tag="bnr")
            nc.vector.reciprocal(rstd, std)
            a_c = sm.tile([128, 1], f32, tag="bna")
            nc.vector.tensor_tensor(out=a_c, in0=rstd, in1=g_col, op=OP.mult)
            d_c = sm.tile([128, 1], f32, tag="bnd")
            nc.vector.tensor_tensor(out=d_c, in0=mean, in1=a_c, op=OP.mult)
            nc.vector.tensor_tensor(out=d_c, in0=b_col, in1=d_c, op=OP.subtract)
            return a_c, d_c

        a2, d2 = bn_finalize(S1, 0, 1, 1.0 / N2, col(0), col(1))
        if debug:
            nc.sync.dma_start(out=dbg["dbg_s1"].ap(), in_=S1)
        for nt in range(2):
            bn_lrelu(H2[:, 2 * nt:2 * nt + 2, 1:17, 1:17],
                     c2p[nt].rearrange("p (s y x) -> p s y x", s=2, x=16),
                     a2[:, 0:1], d2[:, 0:1])
        if debug:
            nc.sync.dma_start(out=dbg["dbg_h1"].ap(), in_=H1)
            nc.sync.dma_start(out=dbg["dbg_h2"].ap(), in_=H2)

        # ---------------- conv3 + BN3 ----------------
        H2v = H2.rearrange("p s (r a) (v b) -> p s r a v b", a=2, b=2)
        c3p = []
        for mc in range(2):
            pt = hold.tile([128, 512], f32, tag="hold")
            c3p.append(pt)
            k = 0
            for dy in range(4):
                for dx in range(4):
                    nc.tensor.matmul(
                        pt[:, 0:256],
                        pk_at("c3w", ((dy * 4 + dx) * 2 + mc) * 128, 128),
                        H2v[:, :, dy // 2:dy // 2 + 8, dy % 2,
                            dx // 2:dx // 2 + 8, dx % 2],
                        start=(k == 0), stop=(k == 15),
                    )
                    k += 1
        AR2 = sm.tile([128, 4], f32, tag="ar2")
        for mc in range(2):
            tb1 = trash.tile([128, 512], f16, tag="tr")
            nc.scalar.activation(out=tb1[:, 0:256], in_=c3p[mc][:, 0:256],
                                 func=AF.Square, accum_out=AR2[:, 2 + mc:3 + mc])
            tb2 = trash.tile([128, 512], f16, tag="tr")
            nc.scalar.activation(out=tb2[:, 0:256], in_=c3p[mc][:, 0:256],
                                 func=AF.Identity, accum_out=AR2[:, mc:mc + 1])
        nc.gpsimd.dma_start(out=cc2_in[:, :], in_=AR2)
        nc.gpsimd.collective_compute(
            "AllReduce", OP.add, replica_groups=groups,
            ins=[cc2_in[:, :]], outs=[cc2_out[:, :]])
        S2 = sm.tile([128, 4], f32, tag="s2")
        nc.gpsimd.dma_start(out=S2, in_=cc2_out[:, :])

        ENC = st.tile([128, 2, bpc, 64], f16)
        for mc in range(2):
            a3, d3 = bn_finalize(S2, mc, 2 + mc, 1.0 / N3, col(2 + mc), col(4 + mc))
            bn_lrelu(ENC[:, mc, :, :],
                     c3p[mc][:, 0:256].rearrange("p (s x) -> p s x", s=bpc),
                     a3[:, 0:1], d3[:, 0:1])
        if debug:
            nc.sync.dma_start(out=dbg["dbg_s2"].ap(), in_=S2)
            nc.sync.dma_start(out=dbg["dbg_enc"].ap(), in_=ENC)

        # ---------------- relation network ----------------
        excl = st.tile([11, 64], f16)
        nc.sync.dma_start(out=excl[10:11, :], in_=pk_at("aro", 0, 64, rows=1))

        bufs = [st.tile([128, 2, NUM * NUM], f16, name=f"rnbuf{i}")
                for i in range(3)]
        POOL = st.tile([128, bpc, 2], f32)

        for s in range(bpc):
            nc.vector.tensor_copy(excl[0:AUX, :],
                                  AUXT[:, s:s + 1].to_broadcast((AUX, 64)))
            acp = hold.tile([128, 512], f32, tag="hold")
            nc.tensor.matmul(acp[0:64, 0:256], ENC[:, 0, s, :],
                             pk_at("w1t", 0, 256), start=True, stop=False)
            nc.tensor.matmul(acp[0:64, 0:256], ENC[:, 1, s, :],
                             pk_at("w1t", 256, 256), start=False, stop=False)
            nc.tensor.matmul(acp[0:64, 0:256], pk_at("aro", 0, 64, rows=2),
                             pk_at("exA", 0, 256, rows=2), start=False, stop=True)
            nc.tensor.matmul(acp[64:128, 0:256], ENC[:, 0, s, :],
                             pk_at("w1b", 0, 256), start=True, stop=False)
            nc.tensor.matmul(acp[64:128, 0:256], ENC[:, 1, s, :],
                             pk_at("w1b", 256, 256), start=False, stop=False)
            nc.tensor.matmul(acp[64:128, 0:256], excl,
                             pk_at("exC", 0, 256, rows=11), start=False, stop=True)
            ac = sm.tile([128, 256], f16, tag="ac")
            nc.scalar.copy(ac, acp[:, 0:256])
            if debug and s == 0:
                nc.sync.dma_start(out=dbg["dbg_ac"].ap(), in_=ac)

            b0, b1, b2 = bufs
            # L1: S.T = AC.T @ G, relu -> b0
            for t2 in range(4):
                for mc in range(2):
                    pt = big.tile([128, 1024], f32, tag="mm")
                    for h in range(2):
                        tt = t2 * 2 + h
                        nc.tensor.matmul(pt[:, h * 512:(h + 1) * 512],
                                         ac[:, mc * 128:(mc + 1) * 128],
                                         G_s[:, tt * 512:(tt + 1) * 512],
                                         start=True, stop=True)
                    relu_bias((t2 * 2 + mc) % 8 < 3,
                              b0[:, mc, t2 * 1024:(t2 + 1) * 1024], pt, None)
            # L2..L4
            for li, (wnm, bci, src, dst) in enumerate((
                    ("gw2", 6, b0, b1), ("gw3", 8, b1, b2), ("gw4", 10, b2, None))):
                if li == 2:
                    p4 = sm.tile([128, 2, 4], f32, tag="p4")
                for t2 in range(4):
                    for mc in range(2):
                        pt = big.tile([128, 1024], f32, tag="mm")
                        for h in range(2):
                            tt = t2 * 2 + h
                            for kc in range(2):
                                nc.tensor.matmul(
                                    pt[:, h * 512:(h + 1) * 512],
                                    pk_at(wnm, (kc * 2 + mc) * 128, 128),
                                    src[:, kc, tt * 512:(tt + 1) * 512],
                                    start=(kc == 0), stop=(kc == 1))
                        if li < 2:
                            relu_bias((t2 * 2 + mc) % 8 < 4,
                                      dst[:, mc, t2 * 1024:(t2 + 1) * 1024], pt,
                                      col(bci + mc))
                        else:
                            # DVE accum_out is broken on HW -> ACT only here
                            tb = trash.tile([128, 1024], f16, tag="tr4")
                            relu_bias(True, tb, pt, col(bci + mc),
                                      accum=p4[:, mc, t2:t2 + 1])
            nc.vector.tensor_reduce(out=POOL[:, s, :], in_=p4,
                                    axis=mybir.AxisListType.X, op=OP.add)
            if debug and s == 0:
                nc.sync.dma_start(out=dbg["dbg_g1"].ap(), in_=b0)
                nc.sync.dma_start(out=dbg["dbg_g3"].ap(), in_=b2)
        if debug:
            nc.sync.dma_start(out=dbg["dbg_pool"].ap(), in_=POOL)

        # ---------------- f MLP ----------------
        POOLh = st.tile([128, bpc, 2], f16)
        nc.vector.tensor_copy(POOLh, POOL)
        F1 = st.tile([128, 2, bpc], f16)
        F2 = st.tile([128, 2, bpc], f16)
        for (wnm, bci, src, dst) in (("fw1", 12, POOLh, F1), ("fw2", 14, F1, F2)):
            for mc in range(2):
                pt = hold.tile([128, 512], f32, tag="hold")
                if src is POOLh:
                    r0, r1 = src[:, :, 0], src[:, :, 1]
                else:
                    r0, r1 = src[:, 0, :], src[:, 1, :]
                nc.tensor.matmul(pt[:, 0:bpc], pk_at(wnm, mc * 128, 128),
                                 r0, start=True, stop=False)
                nc.tensor.matmul(pt[:, 0:bpc], pk_at(wnm, (2 + mc) * 128, 128),
                                 r1, start=False, stop=True)
                relu_bias(True, dst[:, mc, :], pt[:, 0:bpc], col(bci + mc))
        pt = hold.tile([128, 512], f32, tag="hold")
        nc.tensor.matmul(pt[0:1, 0:bpc], pk_at("fw3", 0, 1), F2[:, 0, :],
                         start=True, stop=False)
        nc.tensor.matmul(pt[0:1, 0:bpc], pk_at("fw3", 1, 1), F2[:, 1, :],
                         start=False, stop=True)
        osb = sm.tile([1, bpc], f32, tag="osb")
        nc.scalar.activation(out=osb, in_=pt[0:1, 0:bpc], func=AF.Identity,
                             bias=fb3_s[0:1, 0:1])
        nc.sync.dma_start(out=out[:, :], in_=osb)

    nc.compile()
    return nc


def _in_maps(inputs):
    pk, G, pk32 = _host_pack(inputs)
    img = np.ascontiguousarray(inputs['input_image'], dtype=np.float32)
    aux = np.ascontiguousarray(inputs['inputg_aux_v'], dtype=np.float32)
    in_maps = []
    for c in range(NCORES):
        sl = slice(c * BPC, (c + 1) * BPC)
        p32c = pk32.copy()
        p32c[0:BPC, 161:161 + AUX] = aux[sl]
        in_maps.append({'img': img[sl].copy(), 'pk16': pk, 'Gd': G,
                        'pk32': p32c})
    return in_maps


def kernel(**inputs):
    from concourse.bass_utils import run_bass_kernel_spmd

    key = ("nc", NCORES, BPC)
    if key not in _CACHE:
        _CACHE[key] = _build(NCORES, BPC)
    nc = _CACHE[key]

    in_maps = _in_maps(inputs)
    res = run_bass_kernel_spmd(nc, in_maps, core_ids=list(range(NCORES)))
    kernel.last_result = res
    outs = [res.results[c]["out"].reshape(BPC, 1) for c in range(NCORES)]
    return np.concatenate(outs, axis=0).astype(np.float32)


def bench_steady(inputs, iters=20, nc=None):
    """Steady-state timing of the sharded NEFF execution with device-resident
    inputs (mirrors bass2jax.run_bass_via_pjrt's multi-core path)."""
    import time
    import jax
    import concourse.mybir as mybir
    from concourse import bass2jax
    from jax.sharding import Mesh, PartitionSpec
    from jax.experimental.shard_map import shard_map

    if nc is None:
        key = ("nc", NCORES, BPC)
        if key not in _CACHE:
            _CACHE[key] = _build(NCORES, BPC)
        nc = _CACHE[key]
    in_maps = _in_maps(inputs)

    bass2jax.install_neuronx_cc_hook()
    partition_name = nc.partition_id_tensor.name if nc.partition_id_tensor else None
    in_names, out_names, out_avals, zero_outs = [], [], [], []
    for alloc in nc.m.functions[0].allocations:
        if not isinstance(alloc, mybir.MemoryLocationSet):
            continue
        name = alloc.memorylocations[0].name
        if alloc.kind == "ExternalInput":
            if name != partition_name:
                in_names.append(name)
        elif alloc.kind == "ExternalOutput":
            out_names.append(name)
            shape = tuple(alloc.tensor_shape)
            dtype = mybir.dt.np(alloc.dtype)
            out_avals.append(jax.core.ShapedArray(shape, dtype))
            zero_outs.append(np.zeros(shape, dtype))
    n_params = len(in_names)
    n_outs = len(out_avals)
    all_in_names = list(in_names) + list(out_names)
    if partition_name is not None:
        all_in_names.append(partition_name)

    def _body(*args):
        operands = list(args)
        if partition_name is not None:
            operands.append(bass2jax.partition_id_tensor())
        outs = bass2jax._bass_exec_p.bind(
            *operands,
            out_avals=tuple(out_avals),
            in_names=tuple(all_in_names),
            out_names=tuple(out_names),
            lowering_input_output_aliases=(),
            sim_require_finite=True,
            sim_require_nnan=True,
            nc=nc,
        )
        return tuple(outs)

    devices = jax.devices()[:NCORES]
    mesh = Mesh(np.asarray(devices), ("core",))
    in_specs = (PartitionSpec("core"),) * (n_params + n_outs)
    out_specs = (PartitionSpec("core"),) * len(out_names)
    sharded = jax.jit(
        shard_map(_body, mesh=mesh, in_specs=in_specs, out_specs=out_specs,
                  check_rep=False),
        keep_unused=True,
    )
    concat_in = [
        np.concatenate([np.asarray(in_maps[c][nm]) for c in range(NCORES)], axis=0)
        for nm in in_names
    ]
    concat_zeros = [
        np.zeros((NCORES * z.shape[0], *z.shape[1:]), z.dtype) for z in zero_outs
    ]
    dev_in = [jax.device_put(a) for a in concat_in]
    dev_zero = [jax.device_put(a) for a in concat_zeros]
    r = sharded(*dev_in, *dev_zero)
    jax.block_until_ready(r)
    t0 = time.time()
    for _ in range(iters):
        r = sharded(*dev_in, *dev_zero)
    jax.block_until_ready(r)
    dt = (time.time() - t0) / iters
    return dt, {nm: np.asarray(x) for nm, x in zip(out_names, r)}



# revision 2
# speedup vs baseline: 1.0072x; 1.0072x over previous
"""Trainium2 Bass kernel for nn_DCGAN_C — single-core variant.

All 32 samples run on ONE NeuronCore. Rationale (measured on this axon
tunnel): per-execute dispatch cost is ~2.2ms for 1 device vs ~5-7ms for
multi-device meshes, and it scales with shipped input bytes; the kernel
itself is only ~0.3-0.9ms. Single core also eliminates the BatchNorm
AllReduces entirely (global batch stats become core-local).

Kernel structure:
- conv1 (12->64, 4x4 s2 p1) + conv2 (64->128) fused per sample; im2col
  f16 image DMA'd straight from HBM (input pre-cast to f16 on host).
- BatchNorm (training mode, batch stats over all 32 samples) computed
  locally: per-sample sum/sumsq columns via fused ACT accum_out.
- conv3 (128->256): 16 tap matmuls x 2 output chunks x 2 sample-halves.
- RN layer 1 factorized: pre1[c,i,j] = AT[c,i] + CT[c,j]; AT/CT computed
  directly transposed on the PE (K<=128 matmuls), the 4096-pair
  pre-activation rebuilt by a DVE broadcast-add (stride-0 APs), relu in
  a second (4x-mode) DVE pass. No selector-G matmul, no PSUM traffic.
- RN layers 2-4: dense [4096,256]x[256,256] per sample in fp16; the
  PSUM->SBUF relu passes rotate over ACT/DVE/Pool engines.
- pooled sum over 4096 pairs: fused ACT accum_out columns + tiny reduce.
- f-MLP batched over all 32 samples.
"""
import numpy as np
from contextlib import ExitStack

NCORES = 1
BPC = 32         # samples on the single core
NUM = 64         # spatial positions (8x8)
AUX = 10
EPS = 1e-5

_CACHE = {}

# fp16 const-pack layout: (name, rows, width)
_P16 = [
    ("c1w", 48, 4 * 128),        # [dy*12+ic][dx*128 + oc(dup both halves)]
    ("c2w", 128, 8 * 128),       # [j*64+ic][(dy*2+m)*128 + oc]
    ("c3w", 128, 32 * 128),      # [ic][((dy*4+dx)*2+mc)*128 + oc']
    ("w1t", 128, 2 * 256),       # [p][kc*256 + c]  (gW1 rows 0:256)
    ("w1b", 128, 2 * 256),       # [p][kc*256 + c]  (gW1 rows 257:513)
    ("exA", 2, 256),             # row0 gW1[256] (coord-i), row1 gb1
    ("exC", 11, 256),            # rows 0-9 gW1[513:523] (aux), row10 gW1[523]
    ("aro", 2, 64),              # row0 arange, row1 ones
    ("gw2", 128, 4 * 128),       # [(kc*2+mc)*128 + c]
    ("gw3", 128, 4 * 128),
    ("gw4", 128, 4 * 128),
    ("fw1", 128, 4 * 128),
    ("fw2", 128, 4 * 128),
    ("fw3", 128, 2),
]
_P16_OFF = {}
_off = 0
for _nm, _r, _w in _P16:
    _P16_OFF[_nm] = _off
    _off += _w
P16_W = _off
# fp32 pack [32, 208]: consts rows 0-15 cols 0-128; ident [0:32,128:160];
# fb3 [0,160]; aux [0:32, 161:171]
P32_W = 176


def _host_pack(inputs):
    f16 = np.float16
    w1 = inputs['conv1_w'].astype(np.float32)
    w2 = inputs['conv2_w'].astype(np.float32)
    w3 = inputs['conv3_w'].astype(np.float32)
    gW1 = inputs['gW1'].astype(np.float32)
    gb1 = inputs['gb1'].astype(np.float32)

    pk = np.zeros((128, P16_W), f16)

    def put(nm, arr):
        o = _P16_OFF[nm]
        r, w = arr.shape
        pk[0:r, o:o + w] = arr.astype(f16)

    # conv1: c1w[dy*12+ic, dx*128+oc] with oc duplicated in cols 64..127
    c1 = np.zeros((48, 4 * 128), np.float32)
    t = np.transpose(w1, (3, 2, 1, 0))   # [dx, dy, ic, oc]
    for dx in range(4):
        blk = t[dx].reshape(48, 64)
        c1[:, dx * 128:dx * 128 + 64] = blk
        c1[:, dx * 128 + 64:dx * 128 + 128] = blk
    put("c1w", c1)

    c2 = np.zeros((128, 8 * 128), np.float32)
    for dy in range(4):
        for m in range(2):
            o = (dy * 2 + m) * 128
            for j in range(2):
                c2[j * 64:(j + 1) * 64, o:o + 128] = w2[:, :, dy, 2 * m + j].T
    put("c2w", c2)

    c3 = np.zeros((128, 32 * 128), np.float32)
    for dy in range(4):
        for dx in range(4):
            for mc in range(2):
                o = ((dy * 4 + dx) * 2 + mc) * 128
                c3[:, o:o + 128] = w3[mc * 128:(mc + 1) * 128, :, dy, dx].T
    put("c3w", c3)

    w1t = np.zeros((128, 512), np.float32)
    w1b = np.zeros((128, 512), np.float32)
    for kc in range(2):
        w1t[:, kc * 256:(kc + 1) * 256] = gW1[kc * 128:(kc + 1) * 128]
        w1b[:, kc * 256:(kc + 1) * 256] = gW1[257 + kc * 128:257 + (kc + 1) * 128]
    put("w1t", w1t)
    put("w1b", w1b)
    put("exA", np.stack([gW1[256], gb1], 0))
    put("exC", np.concatenate([gW1[513:523], gW1[523:524]], 0))
    put("aro", np.stack([np.arange(64, dtype=np.float32),
                         np.ones(64, np.float32)], 0))

    def sq(nm, W):
        o = np.zeros((128, 512), np.float32)
        for kc in range(2):
            for mc in range(2):
                o[:, (kc * 2 + mc) * 128:(kc * 2 + mc + 1) * 128] = \
                    W[kc * 128:(kc + 1) * 128, mc * 128:(mc + 1) * 128]
        put(nm, o)

    sq("gw2", inputs['gW2']); sq("gw3", inputs['gW3']); sq("gw4", inputs['gW4'])
    sq("fw1", inputs['fW1']); sq("fw2", inputs['fW2'])
    f3 = np.zeros((128, 2), np.float32)
    for kc in range(2):
        f3[:, kc] = inputs['fW3'][kc * 128:(kc + 1) * 128, 0]
    put("fw3", f3)

    pk32 = np.zeros((32, P32_W), np.float32)
    pk32[0, 0:128] = inputs['bn2_g']; pk32[1, 0:128] = inputs['bn2_b']
    pk32[2, 0:128] = inputs['bn3_g'][:128]; pk32[3, 0:128] = inputs['bn3_g'][128:]
    pk32[4, 0:128] = inputs['bn3_b'][:128]; pk32[5, 0:128] = inputs['bn3_b'][128:]
    pk32[6, 0:128] = inputs['gb2'][:128]; pk32[7, 0:128] = inputs['gb2'][128:]
    pk32[8, 0:128] = inputs['gb3'][:128]; pk32[9, 0:128] = inputs['gb3'][128:]
    pk32[10, 0:128] = inputs['gb4'][:128]; pk32[11, 0:128] = inputs['gb4'][128:]
    pk32[12, 0:128] = inputs['fb1'][:128]; pk32[13, 0:128] = inputs['fb1'][128:]
    pk32[14, 0:128] = inputs['fb2'][:128]; pk32[15, 0:128] = inputs['fb2'][128:]
    pk32[0:32, 128:160] = np.eye(32, dtype=np.float32)
    pk32[0, 160] = float(np.asarray(inputs['fb3']).reshape(-1)[0])
    pk32[0:BPC, 161:161 + AUX] = np.asarray(
        inputs['inputg_aux_v'], dtype=np.float32)
    return pk, pk32


def _build(pk, pk32_arr, bpc=BPC, debug=False, phases=99):
    import concourse.bacc as bacc
    import concourse.mybir as mybir
    import concourse.tile as tile

    f16 = mybir.dt.float16
    f32 = mybir.dt.float32
    AF = mybir.ActivationFunctionType
    OP = mybir.AluOpType

    N2 = bpc * 256
    N3 = bpc * 64

    nc = bacc.Bacc(None, target_bir_lowering=False, num_devices=1)

    img = nc.dram_tensor("img", [bpc, 12, 64, 64], f16, kind="ExternalInput")
    # weights + consts baked into the NEFF (uploaded once at model load,
    # not re-shipped per execute)
    pk16 = nc.inline_tensor(pk, name="pk16")
    pk32 = nc.inline_tensor(pk32_arr, name="pk32")
    out = nc.dram_tensor("out", [1, bpc], f32, kind="ExternalOutput")
    dbg = {}
    if debug:
        for nm, shape, dt_ in (
                ("dbg_h1", [128, 34, 34], f16),        # sample 0 conv1 out
                ("dbg_c2", [128, 2, 256], f16),        # samples 0,1 conv2 pre-BN
                ("dbg_s2", [128, 4], f32),             # bn2 partial sums (s0,s1)
                ("dbg_h2", [128, 2, 18, 18], f16),     # samples 0,1 post-BN2
                ("dbg_enc", [128, 2, 2, 64], f16),     # samples 0,1 ENC
                ("dbg_act", [128, 256], f16),          # sample 0 AT/CT
                ("dbg_b0", [128, 2, 4096], f16),       # sample 0 g1
                ("dbg_b2", [128, 2, 4096], f16),       # sample 0 g3
                ("dbg_pool", [128, 2, 2], f32),        # samples 0,1 pooled
        ):
            dbg[nm] = nc.dram_tensor(nm, shape, dt_, kind="ExternalOutput")

    with tile.TileContext(nc) as tc, ExitStack() as ctx:
        cw = ctx.enter_context(tc.tile_pool(name="cw", bufs=1))
        st = ctx.enter_context(tc.tile_pool(name="st", bufs=1))
        sm = ctx.enter_context(tc.tile_pool(name="sm", bufs=2))
        trash = ctx.enter_context(tc.tile_pool(name="trash", bufs=2))
        prep = ctx.enter_context(tc.tile_pool(name="prep", bufs=2))
        h1p = ctx.enter_context(tc.tile_pool(name="h1p", bufs=3))
        imp = ctx.enter_context(tc.tile_pool(name="imp", bufs=3))
        big = ctx.enter_context(tc.tile_pool(name="big", bufs=3, space="PSUM"))
        sp = ctx.enter_context(tc.tile_pool(name="sp", bufs=2, space="PSUM"))

        PK = cw.tile([128, P16_W], f16, name="PK")
        nc.sync.dma_start(out=PK, in_=pk16.ap())
        P32 = cw.tile([32, P32_W], f32, name="P32")
        nc.sync.dma_start(out=P32, in_=pk32.ap())

        def pk_at(nm, o0, width, rows=128, r0=0):
            o = _P16_OFF[nm] + o0
            return PK[r0:r0 + rows, o:o + width]

        csb = P32[0:20, 0:128]
        id_s = P32[0:32, 128:160]
        fb3_s = P32[0:1, 160:161]
        asb = P32[0:bpc, 161:161 + AUX]

        # consts transpose -> CONST_T [128, 20]
        CONST_T = st.tile([128, 20], f32)
        ptc = sp.tile([128, 256], f32, tag="sp")
        nc.tensor.transpose(ptc[:, 0:20], csb, id_s[0:20, 0:20])
        nc.vector.tensor_copy(CONST_T, ptc[:, 0:20])
        # aux transpose -> AUXT [10, bpc]
        AUXT = st.tile([AUX, bpc], f16)
        pta = sp.tile([128, 256], f32, tag="sp")
        nc.tensor.transpose(pta[0:AUX, 0:bpc], asb, id_s[0:bpc, 0:bpc])
        nc.vector.tensor_copy(AUXT, pta[0:AUX, 0:bpc])

        def col(i):
            return CONST_T[:, i:i + 1]

        # ---- conv1 + conv2 fused, per sample ----
        # IM[dy*12+ic, oy, u] = img[s, ic, 2oy+dy-1, u]
        imgv = img.rearrange("s c (r2 a) w -> c s r2 a w", a=2)
        dma_engines = [nc.sync, nc.gpsimd, nc.scalar]
        DX_ORDER = (1, 2, 0, 3)

        C2 = st.tile([128, bpc, 256], f16)
        S2s = st.tile([128, bpc], f32)
        S2q = st.tile([128, bpc], f32)
        # H2 allocated + zeroed early so the big memset overlaps the conv loop
        H2 = st.tile([128, bpc, 18, 18], f16)
        nc.gpsimd.memset(H2, 0.0)

        h1_tiles = []
        c2_psum = []

        def conv1_emit(s):
            imh = imp.tile([48, 32, 64], f16, tag="imh")
            nc.vector.memset(imh[:, 0:1, :], 0.0)
            nc.vector.memset(imh[:, 31:32, :], 0.0)
            for dy in range(4):
                oy0, nrows = {0: (1, 31), 1: (0, 32), 2: (0, 32), 3: (0, 31)}[dy]
                r20, a = {0: (0, 1), 1: (0, 0), 2: (0, 1), 3: (1, 0)}[dy]
                dma_engines[(s * 4 + dy) % 3].dma_start(
                    out=imh[dy * 12:(dy + 1) * 12, oy0:oy0 + nrows, :],
                    in_=imgv[:, s, r20:r20 + nrows, a, :],
                )
            imv = imh.rearrange("p y (v b) -> p y v b", b=2)  # u = 2v + b

            pt = big.tile([128, 1024], f32, tag="mm")
            ptv = pt.rearrange("p (y x) -> p y x", x=32)
            for oyh in range(2):
                ys = slice(oyh * 16, (oyh + 1) * 16)
                for dx in DX_ORDER:
                    # u = 2ox + dx - 1 = 2v + b
                    if dx == 1:
                        xs, vs, b = slice(0, 32), slice(0, 32), 0
                    elif dx == 2:
                        xs, vs, b = slice(0, 32), slice(0, 32), 1
                    elif dx == 0:
                        xs, vs, b = slice(1, 32), slice(0, 31), 1
                    else:  # dx == 3
                        xs, vs, b = slice(0, 31), slice(1, 32), 0
                    nc.tensor.matmul(
                        ptv[:, ys, xs],
                        pk_at("c1w", dx * 128, 128, rows=48),
                        imv[0:48, ys, vs, b],
                        start=(dx == 1), stop=(dx == 3),
                    )
            # lrelu into zero-padded per-sample H1 tile
            h1 = h1p.tile([128, 34, 34], f16, tag="h1")
            if s < 3:
                nc.vector.memset(h1, 0.0)
            nc.scalar.activation(out=h1[0:64, 1:33, 1:33], in_=ptv[0:64],
                                 func=AF.Prelu, alpha=0.2)
            nc.scalar.activation(out=h1[64:128, 1:33, 0:32], in_=ptv[64:128],
                                 func=AF.Prelu, alpha=0.2)
            h1_tiles.append(h1)
            if debug and s == 0:
                nc.sync.dma_start(out=dbg["dbg_h1"].ap(), in_=h1)

        def conv2_emit(s):
            h1 = h1_tiles[s]
            H1v = h1.rearrange("p (r a) (v b) -> p r a v b", a=2, b=2)
            pt = sp.tile([128, 256], f32, tag="sp")
            k = 0
            for dy in range(4):
                for m in range(2):
                    nc.tensor.matmul(
                        pt,
                        pk_at("c2w", (dy * 2 + m) * 128, 128),
                        H1v[:, dy // 2:dy // 2 + 16, dy % 2, m:m + 16, 0],
                        start=(k == 0), stop=(k == 7),
                    )
                    k += 1
            h1_tiles[s] = None
            # stats + f16 spill: ACT Identity w/ accum (sum), Square w/ accum
            nc.scalar.activation(out=C2[:, s, :], in_=pt, func=AF.Identity,
                                 accum_out=S2s[:, s:s + 1])
            tb = trash.tile([128, 256], f16, tag="tr2")
            nc.scalar.activation(out=tb, in_=pt, func=AF.Square,
                                 accum_out=S2q[:, s:s + 1])

        # software-pipeline conv1/conv2 by one sample
        conv1_emit(0)
        for s in range(1, bpc):
            conv1_emit(s)
            conv2_emit(s - 1)
        conv2_emit(bpc - 1)

        if debug:
            nc.sync.dma_start(out=dbg["dbg_c2"].ap(), in_=C2[:, 0:2, :])
        if phases <= 1:
            nc.sync.dma_start(out=out[:, :], in_=S2s[0:1, 0:bpc])  # phase1 out
            nc.compile()
            return nc

        def bn_finalize(sum_ap, sq_ap, inv_n, g_col, b_col):
            mean = sm.tile([128, 1], f32, tag="bnm")
            nc.vector.tensor_scalar(out=mean, in0=sum_ap,
                                    scalar1=inv_n, scalar2=None, op0=OP.mult)
            var = sm.tile([128, 1], f32, tag="bnv")
            nc.vector.tensor_scalar(out=var, in0=sq_ap,
                                    scalar1=inv_n, scalar2=None, op0=OP.mult)
            msq = sm.tile([128, 1], f32, tag="bnq")
            nc.vector.tensor_tensor(out=msq, in0=mean, in1=mean, op=OP.mult)
            nc.vector.tensor_tensor(out=var, in0=var, in1=msq, op=OP.subtract)
            std = sm.tile([128, 1], f32, tag="bns")
            epsb = sm.tile([128, 1], f32, tag="bne")
            nc.vector.memset(epsb, EPS)
            nc.scalar.activation(out=std, in_=var, func=AF.Sqrt, bias=epsb[:, 0:1])
            rstd = sm.tile([128, 1], f32, tag="bnr")
            nc.vector.reciprocal(rstd, std)
            a_c = sm.tile([128, 1], f32, tag="bna")
            nc.vector.tensor_tensor(out=a_c, in0=rstd, in1=g_col, op=OP.mult)
            d_c = sm.tile([128, 1], f32, tag="bnd")
            nc.vector.tensor_tensor(out=d_c, in0=mean, in1=a_c, op=OP.mult)
            nc.vector.tensor_tensor(out=d_c, in0=b_col, in1=d_c, op=OP.subtract)
            return a_c, d_c

        # ---- BN2 finalize + lrelu -> H2 ----
        S2r = sm.tile([128, 2], f32, tag="s2r")
        nc.vector.tensor_reduce(out=S2r[:, 0:1], in_=S2s,
                                axis=mybir.AxisListType.X, op=OP.add)
        nc.vector.tensor_reduce(out=S2r[:, 1:2], in_=S2q,
                                axis=mybir.AxisListType.X, op=OP.add)
        if debug:
            nc.sync.dma_start(out=dbg["dbg_s2"].ap()[:, 0:2], in_=S2s[:, 0:2])
            nc.sync.dma_start(out=dbg["dbg_s2"].ap()[:, 2:4], in_=S2q[:, 0:2])
        a2, d2 = bn_finalize(S2r[:, 0:1], S2r[:, 1:2], 1.0 / N2, col(0), col(1))

        C2v = C2.rearrange("p s (y x) -> p s y x", x=16)
        for sh in range(2):
            nc.scalar.activation(
                out=H2[:, sh * 16:(sh + 1) * 16, 1:17, 1:17],
                in_=C2v[:, sh * 16:(sh + 1) * 16],
                func=AF.Prelu, bias=d2[:, 0:1], scale=a2[:, 0:1], alpha=0.2)
        if debug:
            nc.sync.dma_start(out=dbg["dbg_h2"].ap(), in_=H2[:, 0:2])

        if phases <= 2:
            nc.gpsimd.dma_start(out=out[:, :], in_=H2[0:1].rearrange('p s y x -> p (s y x)')[:, 0:bpc])
            nc.compile()
            return nc
        # ---- conv3 + BN3 -> ENC ----
        H2v = H2.rearrange("p s (r a) (v b) -> p s r a v b", a=2, b=2)
        C3 = st.tile([128, 2, bpc, 64], f16)
        A3s = st.tile([128, 8], f32)
        A3q = st.tile([128, 8], f32)
        # matmul PSUM output must fit one 2KB bank -> N=512 = 8 samples
        for mc in range(2):
            for sh in range(4):
                ss = slice(sh * 8, (sh + 1) * 8)
                pt = big.tile([128, 1024], f32, tag="mm")
                ptv3 = pt[:, 0:512].rearrange("p (s x) -> p s x", s=8)
                k = 0
                for dy in range(4):
                    for dx in range(4):
                        nc.tensor.matmul(
                            ptv3,
                            pk_at("c3w", ((dy * 4 + dx) * 2 + mc) * 128, 128),
                            H2v[:, ss, dy // 2:dy // 2 + 8, dy % 2,
                                dx // 2:dx // 2 + 8, dx % 2],
                            start=(k == 0), stop=(k == 15),
                        )
                        k += 1
                r = mc * 4 + sh
                nc.scalar.activation(
                    out=C3[:, mc, ss, :].rearrange("p s x -> p (s x)"),
                    in_=pt[:, 0:512], func=AF.Identity,
                    accum_out=A3s[:, r:r + 1])
                tb = trash.tile([128, 1024], f16, tag="tr3")
                nc.scalar.activation(out=tb[:, 0:512], in_=pt[:, 0:512],
                                     func=AF.Square, accum_out=A3q[:, r:r + 1])

        ENC = st.tile([128, 2, bpc, 64], f16)
        A3r = sm.tile([128, 4], f32, tag="a3r")
        for mc in range(2):
            nc.vector.tensor_reduce(
                out=A3r[:, 2 * mc:2 * mc + 1], in_=A3s[:, 4 * mc:4 * mc + 4],
                axis=mybir.AxisListType.X, op=OP.add)
            nc.vector.tensor_reduce(
                out=A3r[:, 2 * mc + 1:2 * mc + 2], in_=A3q[:, 4 * mc:4 * mc + 4],
                axis=mybir.AxisListType.X, op=OP.add)
            a3, d3 = bn_finalize(A3r[:, 2 * mc:2 * mc + 1],
                                 A3r[:, 2 * mc + 1:2 * mc + 2],
                                 1.0 / N3, col(2 + mc), col(4 + mc))
            nc.scalar.activation(
                out=ENC[:, mc].rearrange("p s x -> p (s x)"),
                in_=C3[:, mc].rearrange("p s x -> p (s x)"),
                func=AF.Prelu, bias=d3[:, 0:1], scale=a3[:, 0:1], alpha=0.2)
        if debug:
            nc.sync.dma_start(out=dbg["dbg_enc"].ap(), in_=ENC[:, :, 0:2, :])

        if phases <= 3:
            nc.gpsimd.dma_start(out=out[:, :], in_=ENC[0:1].rearrange('p m s x -> p (m s x)')[:, 0:bpc])
            nc.compile()
            return nc
        # ---------------- relation network ----------------
        excl = st.tile([11, 64], f16)
        nc.sync.dma_start(out=excl[10:11, :], in_=pk_at("aro", 0, 64, rows=1))

        bufs = [st.tile([128, 2, NUM * NUM], f16, name=f"rnbuf{i}")
                for i in range(4)]
        POOL = st.tile([128, bpc, 2], f32)

        # relu-pass engine rotation for L2..L4 PSUM->SBUF drains.
        # L4 must be ACT (accum_out). L2/L3: rotate DVE/Pool/ACT.
        def drain(eng, dst, psum, bias_col, accum=None):
            if eng == "act":
                nc.scalar.activation(out=dst, in_=psum, func=AF.Relu,
                                     bias=bias_col if bias_col is not None else 0.0,
                                     accum_out=accum)
            else:
                e = nc.vector if eng == "dve" else nc.gpsimd
                e.tensor_scalar(
                    out=dst, in0=psum,
                    scalar1=bias_col if bias_col is not None else 0.0,
                    scalar2=0.0, op0=OP.add, op1=OP.max)

        # GPSIMD cannot access PSUM on TRN2 -> drains go to ACT/DVE only.
        # ACT additionally carries all 8 L4 accum drains per sample, so DVE
        # takes the larger share of L2/L3.
        ROT2 = ("dve", "act", "dve", "act", "dve", "act", "dve", "act")
        ROT3 = ("act", "dve", "act", "dve", "act", "dve", "act", "dve")

        def rn_front(s):
            """acp matmuls + act copy + L1 broadcast-add/relu -> b0[s%2]."""
            b0 = bufs[s % 2]
            nc.vector.tensor_copy(excl[0:AUX, :],
                                  AUXT[:, s:s + 1].to_broadcast((AUX, 64)))
            # AT/CT [128c(2mc), 64] computed directly transposed:
            # AT[c,i] = sum_k gW1[k,c] enc[k,i] + i*gW1[256,c] + gb1[c]
            acp = sp.tile([128, 256], f32, tag="sp")
            for mc in range(2):
                # quadrant layout:
                # [0:64) A-mc0, [64:128) A-mc1, [128:192) C-mc0, [192:256) C-mc1
                qa = acp[:, mc * 64:(mc + 1) * 64]
                nc.tensor.matmul(qa, pk_at("w1t", mc * 128, 128),
                                 ENC[:, 0, s, :], start=True, stop=False)
                nc.tensor.matmul(qa, pk_at("w1t", 256 + mc * 128, 128),
                                 ENC[:, 1, s, :], start=False, stop=False)
                nc.tensor.matmul(qa, pk_at("exA", mc * 128, 128, rows=2),
                                 pk_at("aro", 0, 64, rows=2),
                                 start=False, stop=True)
                qc = acp[:, 128 + mc * 64:128 + (mc + 1) * 64]
                nc.tensor.matmul(qc, pk_at("w1b", mc * 128, 128),
                                 ENC[:, 0, s, :], start=True, stop=False)
                nc.tensor.matmul(qc, pk_at("w1b", 256 + mc * 128, 128),
                                 ENC[:, 1, s, :], start=False, stop=False)
                nc.tensor.matmul(qc, pk_at("exC", mc * 128, 128, rows=11),
                                 excl, start=False, stop=True)
            act = sm.tile([128, 256], f16, tag="act")
            nc.scalar.copy(act, acp)
            if debug and s == 0:
                nc.sync.dma_start(out=dbg["dbg_act"].ap(), in_=act)

            # L1: pre[c, i*64+j] = AT[c,i] + CT[c,j] (SBUF broadcast-add,
            # one mc chunk on DVE and one on Pool); relu on DVE (4x mode)
            for mc in range(2):
                at = act[:, mc * 64:(mc + 1) * 64]
                ct = act[:, 128 + mc * 64:128 + (mc + 1) * 64]
                pre = prep.tile([128, 4096], f16, tag="pre")
                prev = pre.rearrange("p (i j) -> p i j", j=64)
                nc.vector.tensor_tensor(
                    out=prev, in0=at.to_broadcast((128, 64, 64)),
                    in1=ct.rearrange("p (a j) -> p a j", a=1)
                        .to_broadcast((128, 64, 64)),
                    op=OP.add)
                nc.vector.tensor_scalar(out=b0[:, mc], in0=pre,
                                        scalar1=0.0, scalar2=None, op0=OP.max)

        def rn_back(s):
            """L2..L4 + pooled sum for sample s (reads b0[s%2])."""
            b0, b1, b2 = bufs[s % 2], bufs[2], bufs[3]
            p4 = None
            for li, (wnm, bci, src, dst, rot) in enumerate((
                    ("gw2", 6, b0, b1, ROT2), ("gw3", 8, b1, b2, ROT3),
                    ("gw4", 10, b2, None, None))):
                if li == 2:
                    p4 = sm.tile([128, 2, 4], f32, tag="p4")
                for t2 in range(4):
                    for mc in range(2):
                        pt = big.tile([128, 1024], f32, tag="mm")
                        for h in range(2):
                            tt = t2 * 2 + h
                            for kc in range(2):
                                nc.tensor.matmul(
                                    pt[:, h * 512:(h + 1) * 512],
                                    pk_at(wnm, (kc * 2 + mc) * 128, 128),
                                    src[:, kc, tt * 512:(tt + 1) * 512],
                                    start=(kc == 0), stop=(kc == 1))
                        if li < 2:
                            drain(rot[t2 * 2 + mc],
                                  dst[:, mc, t2 * 1024:(t2 + 1) * 1024], pt,
                                  col(bci + mc))
                        else:
                            tb = trash.tile([128, 1024], f16, tag="tr4")
                            drain("act", tb, pt, col(bci + mc),
                                  accum=p4[:, mc, t2:t2 + 1])
            nc.vector.tensor_reduce(out=POOL[:, s, :], in_=p4,
                                    axis=mybir.AxisListType.X, op=OP.add)
            if debug and s == 0:
                nc.sync.dma_start(out=dbg["dbg_b0"].ap(), in_=b0)
                nc.sync.dma_start(out=dbg["dbg_b2"].ap(), in_=b2)

        # software-pipeline: sample s+1's front overlaps sample s's L2..L4
        rn_front(0)
        for s in range(bpc):
            if s + 1 < bpc:
                rn_front(s + 1)
            rn_back(s)
        if debug:
            nc.sync.dma_start(out=dbg["dbg_pool"].ap(), in_=POOL[:, 0:2, :])

        # ---------------- f MLP ----------------
        POOLh = st.tile([128, bpc, 2], f16)
        nc.vector.tensor_copy(POOLh, POOL)
        F1 = st.tile([128, 2, bpc], f16)
        F2 = st.tile([128, 2, bpc], f16)
        for (wnm, bci, src, dst) in (("fw1", 12, POOLh, F1), ("fw2", 14, F1, F2)):
            for mc in range(2):
                pt = sp.tile([128, 256], f32, tag="sp")
                if src is POOLh:
                    r0, r1 = src[:, :, 0], src[:, :, 1]
                else:
                    r0, r1 = src[:, 0, :], src[:, 1, :]
                nc.tensor.matmul(pt[:, 0:bpc], pk_at(wnm, mc * 128, 128),
                                 r0, start=True, stop=False)
                nc.tensor.matmul(pt[:, 0:bpc], pk_at(wnm, (2 + mc) * 128, 128),
                                 r1, start=False, stop=True)
                nc.scalar.activation(out=dst[:, mc, :], in_=pt[:, 0:bpc],
                                     func=AF.Relu, bias=col(bci + mc))
        pt = sp.tile([128, 256], f32, tag="sp")
        nc.tensor.matmul(pt[0:1, 0:bpc], pk_at("fw3", 0, 1), F2[:, 0, :],
                         start=True, stop=False)
        nc.tensor.matmul(pt[0:1, 0:bpc], pk_at("fw3", 1, 1), F2[:, 1, :],
                         start=False, stop=True)
        osb = sm.tile([1, bpc], f32, tag="osb")
        nc.scalar.activation(out=osb, in_=pt[0:1, 0:bpc], func=AF.Identity,
                             bias=fb3_s[0:1, 0:1])
        nc.sync.dma_start(out=out[:, :], in_=osb)

    nc.compile()
    return nc


def _get_nc(inputs, debug=False):
    import hashlib
    pk, pk32 = _host_pack(inputs)
    h = hashlib.sha256(pk.tobytes() + pk32.tobytes()).hexdigest()[:16]
    key = ("nc", NCORES, BPC, h, debug)
    if key not in _CACHE:
        _CACHE[key] = _build(pk, pk32, BPC, debug=debug)
    return _CACHE[key]


def _in_map(inputs):
    img = np.ascontiguousarray(inputs['input_image'], dtype=np.float16)
    return {'img': img}


def kernel(**inputs):
    from concourse.bass_utils import run_bass_kernel_spmd

    nc = _get_nc(inputs)
    res = run_bass_kernel_spmd(nc, [_in_map(inputs)], core_ids=[0])
    kernel.last_result = res
    return res.results[0]["out"].reshape(BPC, 1).astype(np.float32)


def bench_steady(inputs, iters=20, nc=None):
    """Steady-state timing of the NEFF execution with device-resident
    inputs (single-core; C++ fast-path dispatch so iterations pipeline)."""
    import time
    import jax
    import concourse.mybir as mybir
    from concourse import bass2jax

    if nc is None:
        nc = _get_nc(inputs)
    in_map = _in_map(inputs)

    bass2jax.install_neuronx_cc_hook()
    partition_name = nc.partition_id_tensor.name if nc.partition_id_tensor else None
    in_names, out_names, out_avals, zero_outs = [], [], [], []
    for alloc in nc.m.functions[0].allocations:
        if not isinstance(alloc, mybir.MemoryLocationSet):
            continue
        name = alloc.memorylocations[0].name
        if alloc.kind == "ExternalInput":
            if name != partition_name:
                in_names.append(name)
        elif alloc.kind == "ExternalOutput":
            out_names.append(name)
            shape = tuple(alloc.tensor_shape)
            dtype = mybir.dt.np(alloc.dtype)
            out_avals.append(jax.core.ShapedArray(shape, dtype))
            zero_outs.append(np.zeros(shape, dtype))
    all_in_names = list(in_names) + list(out_names)
    if partition_name is not None:
        all_in_names.append(partition_name)

    def _body(*args):
        operands = list(args)
        if partition_name is not None:
            operands.append(bass2jax.partition_id_tensor())
        outs = bass2jax._bass_exec_p.bind(
            *operands,
            out_avals=tuple(out_avals),
            in_names=tuple(all_in_names),
            out_names=tuple(out_names),
            lowering_input_output_aliases=(),
            sim_require_finite=True,
            sim_require_nnan=True,
            nc=nc,
        )
        return tuple(outs)

    dev_in = [jax.device_put(np.asarray(in_map[nm])) for nm in in_names]
    dev_zero = [jax.device_put(z) for z in zero_outs]

    def compile_fn():
        return jax.jit(_body, keep_unused=True).lower(
            *dev_in, *dev_zero).compile()

    sharded = bass2jax.fast_dispatch_compile(compile_fn)
    r = sharded(*dev_in, *dev_zero)
    jax.block_until_ready(r)
    t0 = time.time()
    for _ in range(iters):
        r = sharded(*dev_in, *dev_zero)
    jax.block_until_ready(r)
    dt = (time.time() - t0) / iters
    return dt, {nm: np.asarray(x) for nm, x in zip(out_names, r)}
